# revision 25
# baseline (speedup 1.0000x reference)
"""Single-head attention (B=8, S=2048, D=1024) on 8 TRN2 NeuronCores.

Distribution: pure data-parallel over batch — one batch element per core,
no collectives. Each core computes a full [2048, 1024] attention layer.

Key algebraic restructure: softmax over keys j is invariant to adding any
per-query constant r[i], and

  S[i,j] = Q[i]·K[j] = sum_d' H[i,d'] x[j,d'] + r[i]
  with H = x (Wq^T Wk) + (Wk^T bq)   (M = Wq^T Wk precomputed on host)

so the K projection disappears: one projection H (bias w2 = Wk^T bq rides
the per-partition ACT bias) and the scores matmul contracts H against raw
x, which is already resident in SBUF. Per-core matmul rows drop from 950k
to 786k (plus one 512-row ones-matmul per i-tile for the softmax
denominator; the 16-block column-sum pre-reduction runs on the idle DVE).

Per-core dataflow (all-transposed, zero on-chip transposes):
  host supplies xT = x[b].T            [d, s]   (d-major)
                msT = (Wq^T Wk)        [d, d']  (d-major, halves of d')
                wvT = Wv.T             [d, e]
  HT[d',i] = sum_d msT[d,d'] xT[d,i] + w2[d']  (d' on partitions)
  V [j,e]  = sum_d xT[d,j] wvT[d,e]            (j on partitions, bias deferred)
  E [j,i]  = exp((sum_d' xT[d',j] HT[d',i]) / 32)   (scores, transposed)
  acc[p,i] = sum_jb E[jb*128+p, i]              (DVE running sum, bf16 once)
  csum[i]  = sum_p acc[p,i]                     (single ones-matmul, bcast)
  U [e,i]  = sum_j V[j,e] E[j,i]
  O [e,i]  = U[e,i] / csum[i] + bv[e]           (DVE mul + ACT bias epilogue)
  host returns O.T per core, stacked over batch.

Matmuls run in bf16 (f32 PSUM accumulation); rel-err vs f32 reference ~3.5e-3.
"""

import numpy as np

B, S, D = 8, 2048, 1024
P = 128          # partitions
NT = 512         # matmul moving free-dim tile (one PSUM bank in f32)
DB = D // P      # 8 blocks of d/e
JB = S // P      # 16 blocks of j (keys)
ITS = S // NT    # 4 tiles of i (queries)
SCALE = 1.0 / 32.0

_STATE = {}


def _build_nc():
    from contextlib import ExitStack

    import concourse.tile as tile
    from concourse import bacc, mybir

    f32 = mybir.dt.float32
    bf16 = mybir.dt.bfloat16
    AF = mybir.ActivationFunctionType

    nc = bacc.Bacc("TRN2", target_bir_lowering=False, debug=False, num_devices=B)

    # all input streams use 2KB-per-partition-line chunks: a [P, 1024] bf16
    # transfer moves 256KB in the same ~630ns a 1KB-line 128KB chunk takes
    xT = nc.dram_tensor("xT", [2, DB, P, 2 * NT], bf16, kind="ExternalInput").ap()
    msT = nc.dram_tensor("msT", [DB, P, D], bf16, kind="ExternalInput").ap()
    wvT = nc.dram_tensor("wvT", [DB, P, D], bf16, kind="ExternalInput").ap()
    w2_d = nc.dram_tensor("w2", [P, DB], f32, kind="ExternalInput").ap()
    bv_d = nc.dram_tensor("bv", [P, DB], f32, kind="ExternalInput").ap()
    out = nc.dram_tensor("out", [DB, P, S], f32, kind="ExternalOutput").ap()

    def mm_pair(ps_list, lhsT, rhs_list, start, stop):
        for ps, rhs in zip(ps_list, rhs_list):
            nc.tensor.matmul(ps, lhsT=lhsT, rhs=rhs, start=start, stop=stop)

    with tile.TileContext(nc) as tc:
        with ExitStack() as top:
            res = top.enter_context(tc.tile_pool(name="res", bufs=1))
            p1 = top.enter_context(tc.tile_pool(name="p1sbuf", bufs=1))
            p2 = top.enter_context(tc.tile_pool(name="p2sbuf", bufs=2))
            # PSUM: 2+2+3+1 = 8 banks, shared by both phases via fixed tags
            psA = top.enter_context(tc.tile_pool(name="psA", bufs=2, space="PSUM"))
            psB = top.enter_context(tc.tile_pool(name="psB", bufs=2, space="PSUM"))
            psU = top.enter_context(tc.tile_pool(name="psU", bufs=3, space="PSUM"))
            psC = top.enter_context(tc.tile_pool(name="psC", bufs=1, space="PSUM"))

            def ps_cycle():
                while True:
                    yield psA, "pa"
                    yield psB, "pb"
                    yield psU, "pu"

            ps_it = ps_cycle()

            def ps_tile():
                pool, tag = next(ps_it)
                return pool.tile([P, NT], f32, tag=tag, name=tag)

            ht = res.tile([P, DB * S], bf16, tag="ht", name="ht")
            xts = res.tile([P, DB * S], bf16, tag="xts", name="xts")
            vv = res.tile([P, JB * D], bf16, tag="vv", name="vv")
            ones = res.tile([P, NT], bf16, tag="ones", name="ones")
            w2s = res.tile([P, DB], f32, tag="w2s", name="w2s")
            bvs = res.tile([P, DB], f32, tag="bvs", name="bvs")

            nc.vector.memset(ones[:], 1.0)

            # ---------------- phase 1: projections ----------------
            # PE warm-up: dummy matmuls on the ones tile fill the initial
            # DMA wait and lift the HAM clock gate before the first real
            # matmul issues.
            # 6 warmups fill the ~4.5us cold-DMA wait (PE idle anyway) and
            # cross the HAM ~3us continuous-busy threshold, so the first
            # real matmuls start at full clock instead of paying the ramp
            pwm = psC.tile([P, NT], f32, tag="pc", name="pwm")
            NWARM = 6
            for w in range(NWARM):
                nc.tensor.matmul(
                    pwm[:],
                    lhsT=ones[:, 0:P],
                    rhs=ones[:],
                    start=(w == 0),
                    stop=(w == NWARM - 1),
                )

            mss = p1.tile([P, DB * D], bf16, tag="mss", name="mss")
            wvs = p1.tile([P, DB * D], bf16, tag="wvs", name="wvs")
            # sync queue: ms per-db (gates the first H groups), w2 bias
            # (gates the first H ACT ~16us), then the late xT st2/st3 pairs.
            # gpsimd queue: early xT st0/st1 pairs, then wv (consumed only
            # by the V projection ~55us in, so it must sit behind every xT
            # chunk), then bv (phase-2-only).
            nc.gpsimd.dma_start(w2s[:], w2_d)
            for db in range(DB):
                nc.sync.dma_start(mss[:, db * D : (db + 1) * D], msT[db])
            for db in range(DB):
                nc.gpsimd.dma_start(
                    xts[:, db * S : db * S + 2 * NT], xT[0, db]
                )
            for db in range(DB):
                nc.sync.dma_start(
                    xts[:, db * S + 2 * NT : db * S + 4 * NT], xT[1, db]
                )
            for db in range(DB):
                nc.gpsimd.dma_start(wvs[:, db * D : (db + 1) * D], wvT[db])
            nc.gpsimd.dma_start(bvs[:], bv_d)

            # HT: out tiles [128 d', 512 s], accumulate over d blocks.
            # st=0 runs db-outer over eb-groups-of-4 so the PE consumes
            # one fresh ms+xT chunk pair per ~1.5us -- slower than the
            # ~0.7us/chunk DMA delivery -- instead of draining all 8
            # chunks per eb group and stalling on the DMA queues.
            for st in range(ITS):
                if st == 0:
                    # all 8 eb groups concurrent, db-outer: each (ms, xT)
                    # chunk pair feeds 8 matmuls (~1.7us) so the cold DMA
                    # stream (~1.3us per 256KB chunk per queue) stays ahead
                    pqs = [
                        psA.tile([P, NT], f32, tag="pa", name="pq0"),
                        psB.tile([P, NT], f32, tag="pb", name="pq1"),
                        psU.tile([P, NT], f32, tag="pu", name="pq2"),
                        psA.tile([P, NT], f32, tag="pa", name="pq3"),
                        psB.tile([P, NT], f32, tag="pb", name="pq4"),
                        psU.tile([P, NT], f32, tag="pu", name="pq5"),
                        psU.tile([P, NT], f32, tag="pu", name="pq6"),
                        psC.tile([P, NT], f32, tag="pc", name="pq7"),
                    ]
                    for db in range(DB):
                        for eb in range(DB):
                            nc.tensor.matmul(
                                pqs[eb][:],
                                lhsT=mss[:, db * D + eb * P : db * D + (eb + 1) * P],
                                rhs=xts[:, db * S : db * S + NT],
                                start=(db == 0),
                                stop=(db == DB - 1),
                            )
                    for eb in range(DB):
                        nc.scalar.activation(
                            ht[:, eb * S : eb * S + NT],
                            pqs[eb][:],
                            AF.Identity,
                            bias=w2s[:, eb : eb + 1],
                            scale=1.0,
                        )
                    continue
                for eb in range(DB):
                    pqk = ps_tile()
                    for db in range(DB):
                        nc.tensor.matmul(
                            pqk[:],
                            lhsT=mss[:, db * D + eb * P : db * D + (eb + 1) * P],
                            rhs=xts[:, db * S + st * NT : db * S + st * NT + NT],
                            start=(db == 0),
                            stop=(db == DB - 1),
                        )
                    nc.scalar.activation(
                        ht[:, eb * S + st * NT : eb * S + st * NT + NT],
                        pqk[:],
                        AF.Identity,
                        bias=w2s[:, eb : eb + 1],
                        scale=1.0,
                    )

            # V: out tiles [128 j, 512 d']; the two d' tiles share the
            # stationary xT slice.
            for jb in range(JB):
                pva = ps_tile()
                pvb = ps_tile()
                for db in range(DB):
                    mm_pair(
                        [pva[:], pvb[:]],
                        xts[:, db * S + jb * P : db * S + (jb + 1) * P],
                        [
                            wvs[:, db * D : db * D + NT],
                            wvs[:, db * D + NT : db * D + 2 * NT],
                        ],
                        start=(db == 0),
                        stop=(db == DB - 1),
                    )
                nc.vector.tensor_copy(vv[:, jb * D : jb * D + NT], pva[:])
                nc.vector.tensor_copy(vv[:, jb * D + NT : jb * D + 2 * NT], pvb[:])

            # ---------------- phase 2: attention ----------------
            for itp in range(ITS // 2):
                it0, it1 = 2 * itp, 2 * itp + 1
                esba = p2.tile([P, JB * NT], bf16, tag="esba", bufs=1, name="esba")
                esbb = p2.tile([P, JB * NT], bf16, tag="esbb", bufs=1, name="esbb")
                # f32 running per-partition partial sums of the E blocks
                # (DVE), so the softmax denominator needs a single
                # ones-matmul instead of 16.
                acca = p2.tile([P, NT], f32, tag="acca", name="acca")
                accb = p2.tile([P, NT], f32, tag="accb", name="accb")
                accaf = p2.tile([P, NT], bf16, tag="accaf", name="accaf")
                accbf = p2.tile([P, NT], bf16, tag="accbf", name="accbf")
                # scores+exp for both i-tiles; xT slice loaded once
                for jb in range(JB):
                    psea = psA.tile([P, NT], f32, tag="pa", name="psea")
                    pseb = psB.tile([P, NT], f32, tag="pb", name="pseb")
                    for eb in range(DB):
                        mm_pair(
                            [psea[:], pseb[:]],
                            xts[:, eb * S + jb * P : eb * S + (jb + 1) * P],
                            [
                                ht[:, eb * S + it0 * NT : eb * S + (it0 + 1) * NT],
                                ht[:, eb * S + it1 * NT : eb * S + (it1 + 1) * NT],
                            ],
                            start=(eb == 0),
                            stop=(eb == DB - 1),
                        )
                    nc.scalar.activation(
                        esba[:, jb * NT : (jb + 1) * NT], psea[:],
                        AF.Exp, bias=0.0, scale=SCALE,
                    )
                    nc.scalar.activation(
                        esbb[:, jb * NT : (jb + 1) * NT], pseb[:],
                        AF.Exp, bias=0.0, scale=SCALE,
                    )
                    for acc, accf, esb in ((acca, accaf, esba), (accb, accbf, esbb)):
                        blk = esb[:, jb * NT : (jb + 1) * NT]
                        if jb == 0:
                            nc.vector.tensor_copy(acc[:], blk)
                        elif jb < JB - 1:
                            nc.vector.tensor_add(acc[:], acc[:], blk)
                        else:
                            # final add rounds once to bf16 for the
                            # full-rate ones-matmul below
                            nc.vector.tensor_add(accf[:], acc[:], blk)

                for it, esb, accf in ((it0, esba, accaf), (it1, esbb, accbf)):
                    # U db=0 first so the PE has work while the DVE acc
                    # chain and the reciprocal settle
                    psu0 = psU.tile([P, NT], f32, tag="pu", name="psu")
                    for jb in range(JB):
                        nc.tensor.matmul(
                            psu0[:],
                            lhsT=vv[:, jb * D : jb * D + P],
                            rhs=esb[:, jb * NT : (jb + 1) * NT],
                            start=(jb == 0),
                            stop=(jb == JB - 1),
                        )
                    # column sums broadcast to all partitions
                    psb = psC.tile([P, NT], f32, tag="pc", name="psb")
                    nc.tensor.matmul(
                        psb[:], lhsT=ones[:, 0:P], rhs=accf[:], start=True, stop=True
                    )
                    recip = p2.tile([P, NT], f32, tag="recip", name="recip")
                    nc.vector.reciprocal(recip[:], psb[:])

                    for db in range(DB):
                        tmp = p2.tile([P, NT], f32, tag="tmp", bufs=3, name="tmp")
                        osb = p2.tile([P, NT], f32, tag="osb", bufs=3, name="osb")
                        qeng = nc.sync if db % 2 == 0 else nc.gpsimd
                        if it == ITS - 1 and db == DB - 1:
                            # very last tile: accumulate in two 256-column
                            # half-groups so the first half's epilogue and
                            # output DMA overlap the second half's matmuls
                            # and the drain tail is one half-epilogue long
                            hw_ = NT // 2
                            for c in range(2):
                                # one PSUM tile per half: range-sharing one
                                # tile serializes half1's writes behind
                                # half0's epilogue reads
                                psu = psU.tile([P, NT], f32, tag="pu", name="psu")
                                sl = slice(c * hw_, (c + 1) * hw_)
                                for jb in range(JB):
                                    nc.tensor.matmul(
                                        psu[:, sl],
                                        lhsT=vv[:, jb * D + db * P : jb * D + (db + 1) * P],
                                        rhs=esb[:, jb * NT + c * hw_ : jb * NT + (c + 1) * hw_],
                                        start=(jb == 0),
                                        stop=(jb == JB - 1),
                                    )
                                nc.vector.tensor_mul(tmp[:, sl], psu[:, sl], recip[:, sl])
                                nc.scalar.activation(
                                    osb[:, sl],
                                    tmp[:, sl],
                                    AF.Identity,
                                    bias=bvs[:, db : db + 1],
                                    scale=1.0,
                                )
                                qc = nc.sync if c % 2 == 0 else nc.gpsimd
                                qc.dma_start(
                                    out[db, :, it * NT + c * hw_ : it * NT + (c + 1) * hw_],
                                    osb[:, sl],
                                )
                            continue
                        if db == 0:
                            psu = psu0
                        else:
                            psu = psU.tile([P, NT], f32, tag="pu", name="psu")
                            for jb in range(JB):
                                nc.tensor.matmul(
                                    psu[:],
                                    lhsT=vv[:, jb * D + db * P : jb * D + (db + 1) * P],
                                    rhs=esb[:, jb * NT : (jb + 1) * NT],
                                    start=(jb == 0),
                                    stop=(jb == JB - 1),
                                )
                        nc.vector.tensor_mul(tmp[:], psu[:], recip[:])
                        nc.scalar.activation(
                            osb[:],
                            tmp[:],
                            AF.Identity,
                            bias=bvs[:, db : db + 1],
                            scale=1.0,
                        )
                        qeng.dma_start(out[db, :, it * NT : (it + 1) * NT], osb[:])

    nc.compile()
    return nc


def _get_nc():
    if "nc" not in _STATE:
        _STATE["nc"] = _build_nc()
    return _STATE["nc"]


def _prepare_in_maps(x, Wq, bq, Wk, bk, Wv, bv):
    import ml_dtypes

    bf = ml_dtypes.bfloat16
    x = np.asarray(x, dtype=np.float32)
    Wq = np.asarray(Wq, np.float32)
    Wk = np.asarray(Wk, np.float32)
    M = Wq.T @ Wk  # scores bilinear form; softmax absorbs the per-i rest
    ms_h = np.ascontiguousarray(M.reshape(DB, P, D)).astype(bf)
    w2 = Wk.T @ np.asarray(bq, np.float32)
    wv_h = np.ascontiguousarray(np.asarray(Wv, np.float32).T).astype(bf).reshape(DB, P, D)
    w2_h = np.ascontiguousarray(w2.reshape(DB, P).T)
    bv_h = np.ascontiguousarray(np.asarray(bv, np.float32).reshape(DB, P).T)
    in_maps = []
    for b in range(B):
        xt_h = np.ascontiguousarray(
            x[b].T.reshape(DB, P, 2, 2 * NT).transpose(2, 0, 1, 3)
        ).astype(bf)
        in_maps.append(
            {
                "xT": xt_h,
                "msT": ms_h,
                "wvT": wv_h,
                "w2": w2_h,
                "bv": bv_h,
            }
        )
    return in_maps


def _unpack(results):
    out = np.empty((B, S, D), np.float32)
    for b in range(B):
        out[b] = results[b]["out"].reshape(D, S).T
    return out


def kernel(x, Wq, bq, Wk, bk, Wv, bv):
    from concourse.bass_utils import run_bass_kernel_spmd

    nc = _get_nc()
    in_maps = _prepare_in_maps(x, Wq, bq, Wk, bk, Wv, bv)
    last_err = None
    for _attempt in range(3):
        try:
            res = run_bass_kernel_spmd(nc, in_maps, core_ids=list(range(B)))
            return _unpack(res.results)
        except Exception as e:  # transient device errors: retry
            last_err = e
    raise last_err


# revision 26
# speedup vs baseline: 1.0023x; 1.0023x over previous
"""Single-head attention (B=8, S=2048, D=1024) on 8 TRN2 NeuronCores.

Distribution: pure data-parallel over batch — one batch element per core,
no collectives. Each core computes a full [2048, 1024] attention layer.

Key algebraic restructure: softmax over keys j is invariant to adding any
per-query constant r[i], and

  S[i,j] = Q[i]·K[j] = sum_d' H[i,d'] x[j,d'] + r[i]
  with H = x (Wq^T Wk) + (Wk^T bq)   (M = Wq^T Wk precomputed on host)

so the K projection disappears: one projection H (bias w2 = Wk^T bq rides
the per-partition ACT bias) and the scores matmul contracts H against raw
x, which is already resident in SBUF. Per-core matmul rows drop from 950k
to 786k (plus one 512-row ones-matmul per i-tile for the softmax
denominator; the 16-block column-sum pre-reduction runs on the idle DVE).

Per-core dataflow (all-transposed, zero on-chip transposes):
  host supplies xT = x[b].T            [d, s]   (d-major)
                msT = (Wq^T Wk)        [d, d']  (d-major, halves of d')
                wvT = Wv.T             [d, e]
  HT[d',i] = sum_d msT[d,d'] xT[d,i] + w2[d']  (d' on partitions)
  V [j,e]  = sum_d xT[d,j] wvT[d,e]            (j on partitions, bias deferred)
  E [j,i]  = exp((sum_d' xT[d',j] HT[d',i]) / 32)   (scores, transposed)
  acc[p,i] = sum_jb E[jb*128+p, i]              (DVE running sum, bf16 once)
  csum[i]  = sum_p acc[p,i]                     (single ones-matmul, bcast)
  U [e,i]  = sum_j V[j,e] E[j,i]
  O [e,i]  = U[e,i] / csum[i] + bv[e]           (DVE mul + ACT bias epilogue)
  host returns O.T per core, stacked over batch.

Matmuls run in bf16 (f32 PSUM accumulation); rel-err vs f32 reference ~3.5e-3.
"""

import numpy as np

B, S, D = 8, 2048, 1024
P = 128          # partitions
NT = 512         # matmul moving free-dim tile (one PSUM bank in f32)
DB = D // P      # 8 blocks of d/e
JB = S // P      # 16 blocks of j (keys)
ITS = S // NT    # 4 tiles of i (queries)
SCALE = 1.0 / 32.0

_STATE = {}


def _build_nc():
    from contextlib import ExitStack

    import concourse.tile as tile
    from concourse import bacc, mybir

    f32 = mybir.dt.float32
    bf16 = mybir.dt.bfloat16
    AF = mybir.ActivationFunctionType

    nc = bacc.Bacc("TRN2", target_bir_lowering=False, debug=False, num_devices=B)

    # all input streams use 2KB-per-partition-line chunks: a [P, 1024] bf16
    # transfer moves 256KB in the same ~630ns a 1KB-line 128KB chunk takes
    xT = nc.dram_tensor("xT", [2, DB, P, 2 * NT], bf16, kind="ExternalInput").ap()
    msT = nc.dram_tensor("msT", [DB, P, D], bf16, kind="ExternalInput").ap()
    wvT = nc.dram_tensor("wvT", [DB, P, D], bf16, kind="ExternalInput").ap()
    w2_d = nc.dram_tensor("w2", [P, DB], f32, kind="ExternalInput").ap()
    bv_d = nc.dram_tensor("bv", [P, DB], f32, kind="ExternalInput").ap()
    out = nc.dram_tensor("out", [DB, P, S], f32, kind="ExternalOutput").ap()

    def mm_pair(ps_list, lhsT, rhs_list, start, stop):
        for ps, rhs in zip(ps_list, rhs_list):
            nc.tensor.matmul(ps, lhsT=lhsT, rhs=rhs, start=start, stop=stop)

    with tile.TileContext(nc) as tc:
        with ExitStack() as top:
            res = top.enter_context(tc.tile_pool(name="res", bufs=1))
            p1 = top.enter_context(tc.tile_pool(name="p1sbuf", bufs=1))
            p2 = top.enter_context(tc.tile_pool(name="p2sbuf", bufs=2))
            # PSUM: 2+2+3+1 = 8 banks, shared by both phases via fixed tags
            psA = top.enter_context(tc.tile_pool(name="psA", bufs=2, space="PSUM"))
            psB = top.enter_context(tc.tile_pool(name="psB", bufs=2, space="PSUM"))
            psU = top.enter_context(tc.tile_pool(name="psU", bufs=3, space="PSUM"))
            psC = top.enter_context(tc.tile_pool(name="psC", bufs=1, space="PSUM"))

            def ps_cycle():
                while True:
                    yield psA, "pa"
                    yield psB, "pb"
                    yield psU, "pu"

            ps_it = ps_cycle()

            def ps_tile():
                pool, tag = next(ps_it)
                return pool.tile([P, NT], f32, tag=tag, name=tag)

            ht = res.tile([P, DB * S], bf16, tag="ht", name="ht")
            xts = res.tile([P, DB * S], bf16, tag="xts", name="xts")
            vv = res.tile([P, JB * D], bf16, tag="vv", name="vv")
            ones = res.tile([P, NT], bf16, tag="ones", name="ones")
            w2s = res.tile([P, DB], f32, tag="w2s", name="w2s")
            bvs = res.tile([P, DB], f32, tag="bvs", name="bvs")

            nc.vector.memset(ones[:], 1.0)

            # ---------------- phase 1: projections ----------------
            # PE warm-up: dummy matmuls on the ones tile fill the initial
            # DMA wait and lift the HAM clock gate before the first real
            # matmul issues.
            # NWARM=2 only: cold-DMA-ready jitters by ~2us, and any idle gap
            # between warmup end and the first real matmul resets the HAM
            # continuous-busy window anyway, so longer warmups just add time
            pwm = psC.tile([P, NT], f32, tag="pc", name="pwm")
            NWARM = 2
            for w in range(NWARM):
                nc.tensor.matmul(
                    pwm[:],
                    lhsT=ones[:, 0:P],
                    rhs=ones[:],
                    start=(w == 0),
                    stop=(w == NWARM - 1),
                )

            mss = p1.tile([P, DB * D], bf16, tag="mss", name="mss")
            wvs = p1.tile([P, DB * D], bf16, tag="wvs", name="wvs")
            # sync queue: ms per-db (gates the first H groups), w2 bias
            # (gates the first H ACT ~16us), then the late xT st2/st3 pairs.
            # gpsimd queue: early xT st0/st1 pairs, then wv (consumed only
            # by the V projection ~55us in, so it must sit behind every xT
            # chunk), then bv (phase-2-only).
            nc.gpsimd.dma_start(w2s[:], w2_d)
            for db in range(DB):
                nc.sync.dma_start(mss[:, db * D : (db + 1) * D], msT[db])
            for db in range(DB):
                nc.gpsimd.dma_start(
                    xts[:, db * S : db * S + 2 * NT], xT[0, db]
                )
            for db in range(DB):
                nc.sync.dma_start(
                    xts[:, db * S + 2 * NT : db * S + 4 * NT], xT[1, db]
                )
            for db in range(DB):
                nc.gpsimd.dma_start(wvs[:, db * D : (db + 1) * D], wvT[db])
            nc.gpsimd.dma_start(bvs[:], bv_d)

            # HT: out tiles [128 d', 512 s], accumulate over d blocks.
            # st=0 runs db-outer over eb-groups-of-4 so the PE consumes
            # one fresh ms+xT chunk pair per ~1.5us -- slower than the
            # ~0.7us/chunk DMA delivery -- instead of draining all 8
            # chunks per eb group and stalling on the DMA queues.
            for st in range(ITS):
                if st == 0:
                    # all 8 eb groups concurrent, db-outer: each (ms, xT)
                    # chunk pair feeds 8 matmuls (~1.7us) so the cold DMA
                    # stream (~1.3us per 256KB chunk per queue) stays ahead
                    pqs = [
                        psA.tile([P, NT], f32, tag="pa", name="pq0"),
                        psB.tile([P, NT], f32, tag="pb", name="pq1"),
                        psU.tile([P, NT], f32, tag="pu", name="pq2"),
                        psA.tile([P, NT], f32, tag="pa", name="pq3"),
                        psB.tile([P, NT], f32, tag="pb", name="pq4"),
                        psU.tile([P, NT], f32, tag="pu", name="pq5"),
                        psU.tile([P, NT], f32, tag="pu", name="pq6"),
                        psC.tile([P, NT], f32, tag="pc", name="pq7"),
                    ]
                    for db in range(DB):
                        for eb in range(DB):
                            nc.tensor.matmul(
                                pqs[eb][:],
                                lhsT=mss[:, db * D + eb * P : db * D + (eb + 1) * P],
                                rhs=xts[:, db * S : db * S + NT],
                                start=(db == 0),
                                stop=(db == DB - 1),
                            )
                    for eb in range(DB):
                        nc.scalar.activation(
                            ht[:, eb * S : eb * S + NT],
                            pqs[eb][:],
                            AF.Identity,
                            bias=w2s[:, eb : eb + 1],
                            scale=1.0,
                        )
                    continue
                for eb in range(DB):
                    pqk = ps_tile()
                    for db in range(DB):
                        nc.tensor.matmul(
                            pqk[:],
                            lhsT=mss[:, db * D + eb * P : db * D + (eb + 1) * P],
                            rhs=xts[:, db * S + st * NT : db * S + st * NT + NT],
                            start=(db == 0),
                            stop=(db == DB - 1),
                        )
                    nc.scalar.activation(
                        ht[:, eb * S + st * NT : eb * S + st * NT + NT],
                        pqk[:],
                        AF.Identity,
                        bias=w2s[:, eb : eb + 1],
                        scale=1.0,
                    )

            # V: out tiles [128 j, 512 d']; the two d' tiles share the
            # stationary xT slice.
            for jb in range(JB):
                pva = ps_tile()
                pvb = ps_tile()
                for db in range(DB):
                    mm_pair(
                        [pva[:], pvb[:]],
                        xts[:, db * S + jb * P : db * S + (jb + 1) * P],
                        [
                            wvs[:, db * D : db * D + NT],
                            wvs[:, db * D + NT : db * D + 2 * NT],
                        ],
                        start=(db == 0),
                        stop=(db == DB - 1),
                    )
                nc.vector.tensor_copy(vv[:, jb * D : jb * D + NT], pva[:])
                nc.vector.tensor_copy(vv[:, jb * D + NT : jb * D + 2 * NT], pvb[:])

            # ---------------- phase 2: attention ----------------
            for itp in range(ITS // 2):
                it0, it1 = 2 * itp, 2 * itp + 1
                esba = p2.tile([P, JB * NT], bf16, tag="esba", bufs=1, name="esba")
                esbb = p2.tile([P, JB * NT], bf16, tag="esbb", bufs=1, name="esbb")
                # f32 running per-partition partial sums of the E blocks
                # (DVE), so the softmax denominator needs a single
                # ones-matmul instead of 16.
                acca = p2.tile([P, NT], f32, tag="acca", name="acca")
                accb = p2.tile([P, NT], f32, tag="accb", name="accb")
                accaf = p2.tile([P, NT], bf16, tag="accaf", name="accaf")
                accbf = p2.tile([P, NT], bf16, tag="accbf", name="accbf")
                # scores+exp for both i-tiles; xT slice loaded once
                for jb in range(JB):
                    psea = psA.tile([P, NT], f32, tag="pa", name="psea")
                    pseb = psB.tile([P, NT], f32, tag="pb", name="pseb")
                    for eb in range(DB):
                        mm_pair(
                            [psea[:], pseb[:]],
                            xts[:, eb * S + jb * P : eb * S + (jb + 1) * P],
                            [
                                ht[:, eb * S + it0 * NT : eb * S + (it0 + 1) * NT],
                                ht[:, eb * S + it1 * NT : eb * S + (it1 + 1) * NT],
                            ],
                            start=(eb == 0),
                            stop=(eb == DB - 1),
                        )
                    nc.scalar.activation(
                        esba[:, jb * NT : (jb + 1) * NT], psea[:],
                        AF.Exp, bias=0.0, scale=SCALE,
                    )
                    nc.scalar.activation(
                        esbb[:, jb * NT : (jb + 1) * NT], pseb[:],
                        AF.Exp, bias=0.0, scale=SCALE,
                    )
                    for acc, accf, esb in ((acca, accaf, esba), (accb, accbf, esbb)):
                        blk = esb[:, jb * NT : (jb + 1) * NT]
                        if jb == 0:
                            nc.vector.tensor_copy(acc[:], blk)
                        elif jb < JB - 1:
                            nc.vector.tensor_add(acc[:], acc[:], blk)
                        else:
                            # final add rounds once to bf16 for the
                            # full-rate ones-matmul below
                            nc.vector.tensor_add(accf[:], acc[:], blk)

                for it, esb, accf in ((it0, esba, accaf), (it1, esbb, accbf)):
                    # U db=0 first so the PE has work while the DVE acc
                    # chain and the reciprocal settle
                    psu0 = psU.tile([P, NT], f32, tag="pu", name="psu")
                    for jb in range(JB):
                        nc.tensor.matmul(
                            psu0[:],
                            lhsT=vv[:, jb * D : jb * D + P],
                            rhs=esb[:, jb * NT : (jb + 1) * NT],
                            start=(jb == 0),
                            stop=(jb == JB - 1),
                        )
                    # column sums broadcast to all partitions
                    psb = psC.tile([P, NT], f32, tag="pc", name="psb")
                    nc.tensor.matmul(
                        psb[:], lhsT=ones[:, 0:P], rhs=accf[:], start=True, stop=True
                    )
                    recip = p2.tile([P, NT], f32, tag="recip", name="recip")
                    nc.vector.reciprocal(recip[:], psb[:])

                    for db in range(DB):
                        tmp = p2.tile([P, NT], f32, tag="tmp", bufs=3, name="tmp")
                        osb = p2.tile([P, NT], f32, tag="osb", bufs=3, name="osb")
                        qeng = nc.sync if db % 2 == 0 else nc.gpsimd
                        if it == ITS - 1 and db == DB - 1:
                            # very last tile: accumulate in two 256-column
                            # half-groups so the first half's epilogue and
                            # output DMA overlap the second half's matmuls
                            # and the drain tail is one half-epilogue long
                            hw_ = NT // 2
                            for c in range(2):
                                # one PSUM tile per half: range-sharing one
                                # tile serializes half1's writes behind
                                # half0's epilogue reads
                                psu = psU.tile([P, NT], f32, tag="pu", name="psu")
                                sl = slice(c * hw_, (c + 1) * hw_)
                                for jb in range(JB):
                                    nc.tensor.matmul(
                                        psu[:, sl],
                                        lhsT=vv[:, jb * D + db * P : jb * D + (db + 1) * P],
                                        rhs=esb[:, jb * NT + c * hw_ : jb * NT + (c + 1) * hw_],
                                        start=(jb == 0),
                                        stop=(jb == JB - 1),
                                    )
                                nc.vector.tensor_mul(tmp[:, sl], psu[:, sl], recip[:, sl])
                                nc.scalar.activation(
                                    osb[:, sl],
                                    tmp[:, sl],
                                    AF.Identity,
                                    bias=bvs[:, db : db + 1],
                                    scale=1.0,
                                )
                                qc = nc.sync if c % 2 == 0 else nc.gpsimd
                                qc.dma_start(
                                    out[db, :, it * NT + c * hw_ : it * NT + (c + 1) * hw_],
                                    osb[:, sl],
                                )
                            continue
                        if db == 0:
                            psu = psu0
                        else:
                            psu = psU.tile([P, NT], f32, tag="pu", name="psu")
                            for jb in range(JB):
                                nc.tensor.matmul(
                                    psu[:],
                                    lhsT=vv[:, jb * D + db * P : jb * D + (db + 1) * P],
                                    rhs=esb[:, jb * NT : (jb + 1) * NT],
                                    start=(jb == 0),
                                    stop=(jb == JB - 1),
                                )
                        nc.vector.tensor_mul(tmp[:], psu[:], recip[:])
                        nc.scalar.activation(
                            osb[:],
                            tmp[:],
                            AF.Identity,
                            bias=bvs[:, db : db + 1],
                            scale=1.0,
                        )
                        qeng.dma_start(out[db, :, it * NT : (it + 1) * NT], osb[:])

    nc.compile()
    return nc


def _get_nc():
    if "nc" not in _STATE:
        _STATE["nc"] = _build_nc()
    return _STATE["nc"]


def _prepare_in_maps(x, Wq, bq, Wk, bk, Wv, bv):
    import ml_dtypes

    bf = ml_dtypes.bfloat16
    x = np.asarray(x, dtype=np.float32)
    Wq = np.asarray(Wq, np.float32)
    Wk = np.asarray(Wk, np.float32)
    M = Wq.T @ Wk  # scores bilinear form; softmax absorbs the per-i rest
    ms_h = np.ascontiguousarray(M.reshape(DB, P, D)).astype(bf)
    w2 = Wk.T @ np.asarray(bq, np.float32)
    wv_h = np.ascontiguousarray(np.asarray(Wv, np.float32).T).astype(bf).reshape(DB, P, D)
    w2_h = np.ascontiguousarray(w2.reshape(DB, P).T)
    bv_h = np.ascontiguousarray(np.asarray(bv, np.float32).reshape(DB, P).T)
    in_maps = []
    for b in range(B):
        xt_h = np.ascontiguousarray(
            x[b].T.reshape(DB, P, 2, 2 * NT).transpose(2, 0, 1, 3)
        ).astype(bf)
        in_maps.append(
            {
                "xT": xt_h,
                "msT": ms_h,
                "wvT": wv_h,
                "w2": w2_h,
                "bv": bv_h,
            }
        )
    return in_maps


def _unpack(results):
    out = np.empty((B, S, D), np.float32)
    for b in range(B):
        out[b] = results[b]["out"].reshape(D, S).T
    return out


def kernel(x, Wq, bq, Wk, bk, Wv, bv):
    from concourse.bass_utils import run_bass_kernel_spmd

    nc = _get_nc()
    in_maps = _prepare_in_maps(x, Wq, bq, Wk, bk, Wv, bv)
    last_err = None
    for _attempt in range(3):
        try:
            res = run_bass_kernel_spmd(nc, in_maps, core_ids=list(range(B)))
            return _unpack(res.results)
        except Exception as e:  # transient device errors: retry
            last_err = e
    raise last_err


# revision 27
# speedup vs baseline: 1.0034x; 1.0011x over previous
"""Single-head attention (B=8, S=2048, D=1024) on 8 TRN2 NeuronCores.

Distribution: pure data-parallel over batch — one batch element per core,
no collectives. Each core computes a full [2048, 1024] attention layer.

Key algebraic restructure: softmax over keys j is invariant to adding any
per-query constant r[i], and

  S[i,j] = Q[i]·K[j] = sum_d' H[i,d'] x[j,d'] + r[i]
  with H = x (Wq^T Wk) + (Wk^T bq)   (M = Wq^T Wk precomputed on host)

so the K projection disappears: one projection H (bias w2 = Wk^T bq rides
the per-partition ACT bias) and the scores matmul contracts H against raw
x, which is already resident in SBUF. Per-core matmul rows drop from 950k
to 786k (plus one 512-row ones-matmul per i-tile for the softmax
denominator; the 16-block column-sum pre-reduction runs on the idle DVE).

Per-core dataflow (all-transposed, zero on-chip transposes):
  host supplies xT = x[b].T            [d, s]   (d-major)
                msT = (Wq^T Wk)        [d, d']  (d-major, halves of d')
                wvT = Wv.T             [d, e]
  HT[d',i] = sum_d msT[d,d'] xT[d,i] + w2[d']  (d' on partitions)
  V [j,e]  = sum_d xT[d,j] wvT[d,e]            (j on partitions, bias deferred)
  E [j,i]  = exp((sum_d' xT[d',j] HT[d',i]) / 32)   (scores, transposed)
  acc[p,i] = sum_jb E[jb*128+p, i]              (DVE running sum, bf16 once)
  csum[i]  = sum_p acc[p,i]                     (single ones-matmul, bcast)
  U [e,i]  = sum_j V[j,e] E[j,i]
  O [e,i]  = U[e,i] / csum[i] + bv[e]           (DVE mul + ACT bias epilogue)
  host returns O.T per core, stacked over batch.

Matmuls run in bf16 (f32 PSUM accumulation); rel-err vs f32 reference ~3.5e-3.
"""

import numpy as np

B, S, D = 8, 2048, 1024
P = 128          # partitions
NT = 512         # matmul moving free-dim tile (one PSUM bank in f32)
DB = D // P      # 8 blocks of d/e
JB = S // P      # 16 blocks of j (keys)
ITS = S // NT    # 4 tiles of i (queries)
SCALE = 1.0 / 32.0

_STATE = {}


def _build_nc():
    from contextlib import ExitStack

    import concourse.tile as tile
    from concourse import bacc, mybir

    f32 = mybir.dt.float32
    bf16 = mybir.dt.bfloat16
    AF = mybir.ActivationFunctionType

    nc = bacc.Bacc("TRN2", target_bir_lowering=False, debug=False, num_devices=B)

    # all input streams use 2KB-per-partition-line chunks: a [P, 1024] bf16
    # transfer moves 256KB in the same ~630ns a 1KB-line 128KB chunk takes
    xT = nc.dram_tensor("xT", [2, DB, P, 2 * NT], bf16, kind="ExternalInput").ap()
    msT = nc.dram_tensor("msT", [DB, P, D], bf16, kind="ExternalInput").ap()
    wvT = nc.dram_tensor("wvT", [DB, P, D], bf16, kind="ExternalInput").ap()
    w2_d = nc.dram_tensor("w2", [P, DB], f32, kind="ExternalInput").ap()
    bv_d = nc.dram_tensor("bv", [P, DB], f32, kind="ExternalInput").ap()
    out = nc.dram_tensor("out", [DB, P, S], f32, kind="ExternalOutput").ap()

    def mm_pair(ps_list, lhsT, rhs_list, start, stop):
        for ps, rhs in zip(ps_list, rhs_list):
            nc.tensor.matmul(ps, lhsT=lhsT, rhs=rhs, start=start, stop=stop)

    with tile.TileContext(nc) as tc:
        with ExitStack() as top:
            res = top.enter_context(tc.tile_pool(name="res", bufs=1))
            p1 = top.enter_context(tc.tile_pool(name="p1sbuf", bufs=1))
            p2 = top.enter_context(tc.tile_pool(name="p2sbuf", bufs=2))
            # PSUM: 2+2+3+1 = 8 banks, shared by both phases via fixed tags
            psA = top.enter_context(tc.tile_pool(name="psA", bufs=2, space="PSUM"))
            psB = top.enter_context(tc.tile_pool(name="psB", bufs=2, space="PSUM"))
            psU = top.enter_context(tc.tile_pool(name="psU", bufs=3, space="PSUM"))
            psC = top.enter_context(tc.tile_pool(name="psC", bufs=1, space="PSUM"))

            def ps_cycle():
                while True:
                    yield psA, "pa"
                    yield psB, "pb"
                    yield psU, "pu"

            ps_it = ps_cycle()

            def ps_tile():
                pool, tag = next(ps_it)
                return pool.tile([P, NT], f32, tag=tag, name=tag)

            ht = res.tile([P, DB * S], bf16, tag="ht", name="ht")
            xts = res.tile([P, DB * S], bf16, tag="xts", name="xts")
            vv = res.tile([P, JB * D], bf16, tag="vv", name="vv")
            ones = res.tile([P, NT], bf16, tag="ones", name="ones")
            w2s = res.tile([P, DB], f32, tag="w2s", name="w2s")
            bvs = res.tile([P, DB], f32, tag="bvs", name="bvs")

            nc.vector.memset(ones[:], 1.0)

            # ---------------- phase 1: projections ----------------
            # PE warm-up: dummy matmuls on the ones tile fill the initial
            # DMA wait and lift the HAM clock gate before the first real
            # matmul issues.
            # NWARM=2 only: cold-DMA-ready jitters by ~2us, and any idle gap
            # between warmup end and the first real matmul resets the HAM
            # continuous-busy window anyway, so longer warmups just add time
            pwm = psC.tile([P, NT], f32, tag="pc", name="pwm")
            NWARM = 2
            for w in range(NWARM):
                nc.tensor.matmul(
                    pwm[:],
                    lhsT=ones[:, 0:P],
                    rhs=ones[:],
                    start=(w == 0),
                    stop=(w == NWARM - 1),
                )

            # one ms tile per db so a reader depends only on its own
            # chunk write, not on all eight ms DMAs
            mst = [
                p1.tile([P, D], bf16, tag=f"ms{db}", name=f"ms{db}")
                for db in range(DB)
            ]
            wvs = p1.tile([P, DB * D], bf16, tag="wvs", name="wvs")
            # sync queue: ms per-db (gates the first H groups), w2 bias
            # (gates the first H ACT ~16us), then the late xT st2/st3 pairs.
            # gpsimd queue: early xT st0/st1 pairs, then wv (consumed only
            # by the V projection ~55us in, so it must sit behind every xT
            # chunk), then bv (phase-2-only).
            nc.gpsimd.dma_start(w2s[:], w2_d)
            for db in range(DB):
                nc.sync.dma_start(mst[db][:], msT[db])
            for db in range(DB):
                nc.gpsimd.dma_start(
                    xts[:, db * S : db * S + 2 * NT], xT[0, db]
                )
            for db in range(DB):
                nc.sync.dma_start(
                    xts[:, db * S + 2 * NT : db * S + 4 * NT], xT[1, db]
                )
            for db in range(DB):
                nc.gpsimd.dma_start(wvs[:, db * D : (db + 1) * D], wvT[db])
            nc.gpsimd.dma_start(bvs[:], bv_d)

            # HT: out tiles [128 d', 512 s], accumulate over d blocks.
            # st=0 runs db-outer over eb-groups-of-4 so the PE consumes
            # one fresh ms+xT chunk pair per ~1.5us -- slower than the
            # ~0.7us/chunk DMA delivery -- instead of draining all 8
            # chunks per eb group and stalling on the DMA queues.
            for st in range(ITS):
                if st == 0:
                    # all 8 eb groups concurrent, db-outer: each (ms, xT)
                    # chunk pair feeds 8 matmuls (~1.7us) so the cold DMA
                    # stream (~1.3us per 256KB chunk per queue) stays ahead
                    pqs = [
                        psA.tile([P, NT], f32, tag="pa", name="pq0"),
                        psB.tile([P, NT], f32, tag="pb", name="pq1"),
                        psU.tile([P, NT], f32, tag="pu", name="pq2"),
                        psA.tile([P, NT], f32, tag="pa", name="pq3"),
                        psB.tile([P, NT], f32, tag="pb", name="pq4"),
                        psU.tile([P, NT], f32, tag="pu", name="pq5"),
                        psU.tile([P, NT], f32, tag="pu", name="pq6"),
                        psC.tile([P, NT], f32, tag="pc", name="pq7"),
                    ]
                    for db in range(DB):
                        for eb in range(DB):
                            nc.tensor.matmul(
                                pqs[eb][:],
                                lhsT=mst[db][:, eb * P : (eb + 1) * P],
                                rhs=xts[:, db * S : db * S + NT],
                                start=(db == 0),
                                stop=(db == DB - 1),
                            )
                    for eb in range(DB):
                        nc.scalar.activation(
                            ht[:, eb * S : eb * S + NT],
                            pqs[eb][:],
                            AF.Identity,
                            bias=w2s[:, eb : eb + 1],
                            scale=1.0,
                        )
                    continue
                for eb in range(DB):
                    pqk = ps_tile()
                    for db in range(DB):
                        nc.tensor.matmul(
                            pqk[:],
                            lhsT=mst[db][:, eb * P : (eb + 1) * P],
                            rhs=xts[:, db * S + st * NT : db * S + st * NT + NT],
                            start=(db == 0),
                            stop=(db == DB - 1),
                        )
                    nc.scalar.activation(
                        ht[:, eb * S + st * NT : eb * S + st * NT + NT],
                        pqk[:],
                        AF.Identity,
                        bias=w2s[:, eb : eb + 1],
                        scale=1.0,
                    )

            # V: out tiles [128 j, 512 d']; the two d' tiles share the
            # stationary xT slice.
            for jb in range(JB):
                pva = ps_tile()
                pvb = ps_tile()
                for db in range(DB):
                    mm_pair(
                        [pva[:], pvb[:]],
                        xts[:, db * S + jb * P : db * S + (jb + 1) * P],
                        [
                            wvs[:, db * D : db * D + NT],
                            wvs[:, db * D + NT : db * D + 2 * NT],
                        ],
                        start=(db == 0),
                        stop=(db == DB - 1),
                    )
                nc.vector.tensor_copy(vv[:, jb * D : jb * D + NT], pva[:])
                nc.vector.tensor_copy(vv[:, jb * D + NT : jb * D + 2 * NT], pvb[:])

            # ---------------- phase 2: attention ----------------
            for itp in range(ITS // 2):
                it0, it1 = 2 * itp, 2 * itp + 1
                esba = p2.tile([P, JB * NT], bf16, tag="esba", bufs=1, name="esba")
                esbb = p2.tile([P, JB * NT], bf16, tag="esbb", bufs=1, name="esbb")
                # f32 running per-partition partial sums of the E blocks
                # (DVE), so the softmax denominator needs a single
                # ones-matmul instead of 16.
                acca = p2.tile([P, NT], f32, tag="acca", name="acca")
                accb = p2.tile([P, NT], f32, tag="accb", name="accb")
                accaf = p2.tile([P, NT], bf16, tag="accaf", name="accaf")
                accbf = p2.tile([P, NT], bf16, tag="accbf", name="accbf")
                # scores+exp for both i-tiles; xT slice loaded once
                for jb in range(JB):
                    psea = psA.tile([P, NT], f32, tag="pa", name="psea")
                    pseb = psB.tile([P, NT], f32, tag="pb", name="pseb")
                    for eb in range(DB):
                        mm_pair(
                            [psea[:], pseb[:]],
                            xts[:, eb * S + jb * P : eb * S + (jb + 1) * P],
                            [
                                ht[:, eb * S + it0 * NT : eb * S + (it0 + 1) * NT],
                                ht[:, eb * S + it1 * NT : eb * S + (it1 + 1) * NT],
                            ],
                            start=(eb == 0),
                            stop=(eb == DB - 1),
                        )
                    nc.scalar.activation(
                        esba[:, jb * NT : (jb + 1) * NT], psea[:],
                        AF.Exp, bias=0.0, scale=SCALE,
                    )
                    nc.scalar.activation(
                        esbb[:, jb * NT : (jb + 1) * NT], pseb[:],
                        AF.Exp, bias=0.0, scale=SCALE,
                    )
                    for acc, accf, esb in ((acca, accaf, esba), (accb, accbf, esbb)):
                        blk = esb[:, jb * NT : (jb + 1) * NT]
                        if jb == 0:
                            nc.vector.tensor_copy(acc[:], blk)
                        elif jb < JB - 1:
                            nc.vector.tensor_add(acc[:], acc[:], blk)
                        else:
                            # final add rounds once to bf16 for the
                            # full-rate ones-matmul below
                            nc.vector.tensor_add(accf[:], acc[:], blk)

                for it, esb, accf in ((it0, esba, accaf), (it1, esbb, accbf)):
                    # U db=0 first so the PE has work while the DVE acc
                    # chain and the reciprocal settle
                    psu0 = psU.tile([P, NT], f32, tag="pu", name="psu")
                    for jb in range(JB):
                        nc.tensor.matmul(
                            psu0[:],
                            lhsT=vv[:, jb * D : jb * D + P],
                            rhs=esb[:, jb * NT : (jb + 1) * NT],
                            start=(jb == 0),
                            stop=(jb == JB - 1),
                        )
                    # column sums broadcast to all partitions
                    psb = psC.tile([P, NT], f32, tag="pc", name="psb")
                    nc.tensor.matmul(
                        psb[:], lhsT=ones[:, 0:P], rhs=accf[:], start=True, stop=True
                    )
                    recip = p2.tile([P, NT], f32, tag="recip", name="recip")
                    nc.vector.reciprocal(recip[:], psb[:])

                    for db in range(DB):
                        tmp = p2.tile([P, NT], f32, tag="tmp", bufs=3, name="tmp")
                        osb = p2.tile([P, NT], f32, tag="osb", bufs=3, name="osb")
                        qeng = nc.sync if db % 2 == 0 else nc.gpsimd
                        if it == ITS - 1 and db == DB - 1:
                            # very last tile: accumulate in two 256-column
                            # half-groups so the first half's epilogue and
                            # output DMA overlap the second half's matmuls
                            # and the drain tail is one half-epilogue long
                            hw_ = NT // 2
                            for c in range(2):
                                # one PSUM tile per half: range-sharing one
                                # tile serializes half1's writes behind
                                # half0's epilogue reads
                                psu = psU.tile([P, NT], f32, tag="pu", name="psu")
                                sl = slice(c * hw_, (c + 1) * hw_)
                                for jb in range(JB):
                                    nc.tensor.matmul(
                                        psu[:, sl],
                                        lhsT=vv[:, jb * D + db * P : jb * D + (db + 1) * P],
                                        rhs=esb[:, jb * NT + c * hw_ : jb * NT + (c + 1) * hw_],
                                        start=(jb == 0),
                                        stop=(jb == JB - 1),
                                    )
                                nc.vector.tensor_mul(tmp[:, sl], psu[:, sl], recip[:, sl])
                                nc.scalar.activation(
                                    osb[:, sl],
                                    tmp[:, sl],
                                    AF.Identity,
                                    bias=bvs[:, db : db + 1],
                                    scale=1.0,
                                )
                                qc = nc.sync if c % 2 == 0 else nc.gpsimd
                                qc.dma_start(
                                    out[db, :, it * NT + c * hw_ : it * NT + (c + 1) * hw_],
                                    osb[:, sl],
                                )
                            continue
                        if db == 0:
                            psu = psu0
                        else:
                            psu = psU.tile([P, NT], f32, tag="pu", name="psu")
                            for jb in range(JB):
                                nc.tensor.matmul(
                                    psu[:],
                                    lhsT=vv[:, jb * D + db * P : jb * D + (db + 1) * P],
                                    rhs=esb[:, jb * NT : (jb + 1) * NT],
                                    start=(jb == 0),
                                    stop=(jb == JB - 1),
                                )
                        nc.vector.tensor_mul(tmp[:], psu[:], recip[:])
                        nc.scalar.activation(
                            osb[:],
                            tmp[:],
                            AF.Identity,
                            bias=bvs[:, db : db + 1],
                            scale=1.0,
                        )
                        qeng.dma_start(out[db, :, it * NT : (it + 1) * NT], osb[:])

    nc.compile()
    return nc


def _get_nc():
    if "nc" not in _STATE:
        _STATE["nc"] = _build_nc()
    return _STATE["nc"]


def _prepare_in_maps(x, Wq, bq, Wk, bk, Wv, bv):
    import ml_dtypes

    bf = ml_dtypes.bfloat16
    x = np.asarray(x, dtype=np.float32)
    Wq = np.asarray(Wq, np.float32)
    Wk = np.asarray(Wk, np.float32)
    M = Wq.T @ Wk  # scores bilinear form; softmax absorbs the per-i rest
    ms_h = np.ascontiguousarray(M.reshape(DB, P, D)).astype(bf)
    w2 = Wk.T @ np.asarray(bq, np.float32)
    wv_h = np.ascontiguousarray(np.asarray(Wv, np.float32).T).astype(bf).reshape(DB, P, D)
    w2_h = np.ascontiguousarray(w2.reshape(DB, P).T)
    bv_h = np.ascontiguousarray(np.asarray(bv, np.float32).reshape(DB, P).T)
    in_maps = []
    for b in range(B):
        xt_h = np.ascontiguousarray(
            x[b].T.reshape(DB, P, 2, 2 * NT).transpose(2, 0, 1, 3)
        ).astype(bf)
        in_maps.append(
            {
                "xT": xt_h,
                "msT": ms_h,
                "wvT": wv_h,
                "w2": w2_h,
                "bv": bv_h,
            }
        )
    return in_maps


def _unpack(results):
    out = np.empty((B, S, D), np.float32)
    for b in range(B):
        out[b] = results[b]["out"].reshape(D, S).T
    return out


def kernel(x, Wq, bq, Wk, bk, Wv, bv):
    from concourse.bass_utils import run_bass_kernel_spmd

    nc = _get_nc()
    in_maps = _prepare_in_maps(x, Wq, bq, Wk, bk, Wv, bv)
    last_err = None
    for _attempt in range(3):
        try:
            res = run_bass_kernel_spmd(nc, in_maps, core_ids=list(range(B)))
            return _unpack(res.results)
        except Exception as e:  # transient device errors: retry
            last_err = e
    raise last_err
